# revision 5
# baseline (speedup 1.0000x reference)
"""DenseStructuralGAT layer on 8 Trainium2 NeuronCores.

Row-parallel sharding: core c owns rows [c*1024, (c+1)*1024) of the
8192x8192 attention problem.

Launch 1 (tiny): each core computes its slice H_c = X_c @ W^T (fp16 out),
s_c = X_c @ (W^T a_l), t_c = X_c @ (W^T a_r)  (fp32).
Host concatenates H (fp16) and t (fp32) - pure data movement.

Launch 2 (main): per core, over its 1024x8192 block of M:
  e     = prelu(t_j + s_i, alpha=0.2)            [ACT, one pass, fp16]
  v     = fp16(M) * e                            [DVE tensor_tensor]
  z     = (fp16(M) <= 0) * -50                   [GPSIMD dual-op tensor_scalar]
  w     = v + z                                  [DVE tensor_tensor]
  p     = exp(w), rowsum += accum_out            [ACT, fp16 out]
  pT    = transpose(p) via PE identity matmuls   [PE -> PSUM -> SBUF fp16]
  Z^T  += H_tile^T @ pT                          [PE, fp32 PSUM accum]
  Z     = sigmoid(Z^T.T * (1/rowsum))            [PE transpose + ACT]

Masked entries (M==0) get logits=-50 -> p ~ 2e-22, i.e. softmax-identical
to the reference's -inf masking. Rows with no neighbours do not occur for
this input distribution (checked in test.py; p(no-neighbour) ~ 0.9^8192).
"""

import numpy as np
import ml_dtypes

import concourse.bacc as bacc
import concourse.mybir as mybir
import concourse.tile as tile
from concourse.bass_utils import run_bass_kernel_spmd
from concourse.masks import make_identity

N, FIN, FOUT = 8192, 512, 256
NCORES = 8
RB = N // NCORES          # 1024 rows per core
NIT = RB // 128           # 8 i-tiles per core
NJC = 8                   # j-chunks
JC = N // NJC             # 1024 columns per chunk
NJS = JC // 128           # 8 j-subtiles per chunk
NJT = N // 128            # 64 j-tiles total
NEG = -50.0
ALPHA = 0.2

F32 = mybir.dt.float32
F16 = mybir.dt.float16
AF = mybir.ActivationFunctionType
ALU = mybir.AluOpType


# ----------------------------------------------------------------- launch 1

def build_h_kernel():
    nc = bacc.Bacc()
    x_d = nc.dram_tensor("xc", [RB, FIN], F32, kind="ExternalInput")
    w_d = nc.dram_tensor("w", [FOUT, FIN], F32, kind="ExternalInput")
    a_d = nc.dram_tensor("attn", [1, 2 * FOUT], F32, kind="ExternalInput")
    h_d = nc.dram_tensor("hc", [RB, FOUT], F16, kind="ExternalOutput")
    s_d = nc.dram_tensor("sc", [RB, 1], F32, kind="ExternalOutput")
    t_d = nc.dram_tensor("tc", [RB, 1], F32, kind="ExternalOutput")

    with tile.TileContext(nc) as tc:
        with (
            tc.tile_pool(name="sb", bufs=1) as sb,
            tc.tile_pool(name="ps", bufs=2, space="PSUM") as ps,
            tc.tile_pool(name="psacc", bufs=2, space="PSUM") as psacc,
        ):
            ident = sb.tile([128, 128], F32, tag="ident")
            make_identity(nc, ident[:])
            one11 = sb.tile([1, 1], F32, tag="one")
            nc.vector.memset(one11[:], 1.0)

            x_sb = sb.tile([128, NIT, FIN], F32, tag="xsb")
            nc.sync.dma_start(
                x_sb[:], x_d[:].rearrange("(it p) k -> p it k", p=128)
            )
            w_sb = sb.tile([128, 2, FIN], F32, tag="wsb")
            nc.sync.dma_start(
                w_sb[:], w_d[:].rearrange("(ft p) k -> p ft k", p=128)
            )
            a_sb = sb.tile([1, 2 * FOUT], F32, tag="asb")
            nc.sync.dma_start(a_sb[:], a_d[:])

            # attention vector chunks as columns: 4 chunks of 128
            # [a_l0 a_l1 | a_r0 a_r1]
            a_cols = sb.tile([128, 4], F32, tag="acols")
            for h in range(4):
                pa = ps.tile([128, 1], F32, tag="pt")
                nc.tensor.matmul(pa[:], a_sb[0:1, 128 * h:128 * (h + 1)], one11[:])
                nc.any.tensor_copy(a_cols[:, h:h + 1], pa[:])

            # W^T tiles: WT[kt] = (128k x 256f)
            wt_sb = sb.tile([128, 4 * FOUT], F32, tag="wtsb")
            for kt in range(4):
                for ft in range(2):
                    pw = ps.tile([128, 128], F32, tag="pt")
                    nc.tensor.matmul(
                        pw[:],
                        w_sb[:, ft, kt * 128:(kt + 1) * 128],
                        ident[:],
                    )
                    nc.any.tensor_copy(
                        wt_sb[:, kt * FOUT + ft * 128: kt * FOUT + (ft + 1) * 128],
                        pw[:],
                    )

            # w_s / w_t columns per k-tile: wst[kc] = (128k x 2)
            wst = sb.tile([128, 8], F32, tag="wst")
            for kc in range(4):
                pst = psacc.tile([128, 2], F32, tag="pstc")
                for ft in range(2):
                    rhs = sb.tile([128, 2], F32, tag="arhs")
                    nc.vector.tensor_copy(rhs[:, 0:1], a_cols[:, ft:ft + 1])
                    nc.vector.tensor_copy(rhs[:, 1:2], a_cols[:, 2 + ft:3 + ft])
                    nc.tensor.matmul(
                        pst[:],
                        w_sb[:, ft, kc * 128:(kc + 1) * 128],
                        rhs[:],
                        start=(ft == 0),
                        stop=(ft == 1),
                    )
                nc.any.tensor_copy(wst[:, 2 * kc: 2 * kc + 2], pst[:])

            # X^T tiles: XT[kt] = (128k x 1024i)
            xt_sb = sb.tile([128, 4 * RB], F32, tag="xtsb")
            for it in range(NIT):
                for kt in range(4):
                    px = ps.tile([128, 128], F32, tag="pt")
                    nc.tensor.matmul(
                        px[:],
                        x_sb[:, it, kt * 128:(kt + 1) * 128],
                        ident[:],
                    )
                    nc.any.tensor_copy(
                        xt_sb[:, kt * RB + it * 128: kt * RB + (it + 1) * 128],
                        px[:],
                    )

            # H_c, s_c, t_c
            for it in range(NIT):
                ph = psacc.tile([128, FOUT], F32, tag="ph")
                pstc = psacc.tile([128, 2], F32, tag="pstc")
                for kt in range(4):
                    lhs = xt_sb[:, kt * RB + it * 128: kt * RB + (it + 1) * 128]
                    nc.tensor.matmul(
                        ph[:], lhs, wt_sb[:, kt * FOUT:(kt + 1) * FOUT],
                        start=(kt == 0), stop=(kt == 3),
                    )
                    nc.tensor.matmul(
                        pstc[:], lhs, wst[:, 2 * kt: 2 * kt + 2],
                        start=(kt == 0), stop=(kt == 3),
                    )
                hb = sb.tile([128, FOUT], F16, tag="hb")
                stb = sb.tile([128, 2], F32, tag="stb")
                nc.any.tensor_copy(hb[:], ph[:])
                nc.any.tensor_copy(stb[:], pstc[:])
                nc.sync.dma_start(h_d[it * 128:(it + 1) * 128, :], hb[:])
                nc.sync.dma_start(s_d[it * 128:(it + 1) * 128, :], stb[:, 0:1])
                nc.sync.dma_start(t_d[it * 128:(it + 1) * 128, :], stb[:, 1:2])
    nc.finalize()
    return nc


# ----------------------------------------------------------------- launch 2

def build_main_kernel():
    nc = bacc.Bacc()
    m_d = nc.dram_tensor("mc", [RB, N], F32, kind="ExternalInput")
    h_d = nc.dram_tensor("hf", [N, FOUT], F16, kind="ExternalInput")
    t_d = nc.dram_tensor("tf", [1, N], F32, kind="ExternalInput")
    s_d = nc.dram_tensor("sc", [RB, 1], F32, kind="ExternalInput")
    z_d = nc.dram_tensor("zc", [RB, FOUT], F32, kind="ExternalOutput")

    with tile.TileContext(nc) as tc:
        with (
            tc.tile_pool(name="const", bufs=1) as cst,
            tc.tile_pool(name="zps", bufs=1, space="PSUM") as zpool,
            tc.tile_pool(name="trps", bufs=2, space="PSUM") as trpool,
            tc.tile_pool(name="mf", bufs=6) as mfp,
            tc.tile_pool(name="chain", bufs=3) as chp,
            tc.tile_pool(name="pchain", bufs=NIT + 2) as ppool,
            tc.tile_pool(name="misc", bufs=2) as misc,
        ):
            ident16 = cst.tile([128, 128], F16, tag="id16")
            make_identity(nc, ident16[:])
            ident32 = cst.tile([128, 128], F32, tag="id32")
            make_identity(nc, ident32[:])
            ones1 = cst.tile([1, 128], F32, tag="ones1")
            nc.vector.memset(ones1[:], 1.0)

            # full H as lhsT tiles: (128j x 64jt*256f) fp16
            h_sb = cst.tile([128, NJT, FOUT], F16, tag="hsb")
            nc.sync.dma_start(
                h_sb[:], h_d[:].rearrange("(jt p) f -> p jt f", p=128)
            )
            # s columns (128 x 8it)
            s_sb = cst.tile([128, NIT, 1], F32, tag="ssb")
            nc.sync.dma_start(
                s_sb[:], s_d[:].rearrange("(it p) o -> p it o", p=128)
            )
            # t row
            t_row = cst.tile([1, N], F32, tag="trow")
            nc.sync.dma_start(t_row[:], t_d[:])
            # T_bcast (128 x 8192) fp16 via K=1 ones matmul
            t_sb = cst.tile([128, N], F16, tag="tsb")
            for ch in range(N // 512):
                pt = trpool.tile([128, 512], F32, tag="pstr")
                nc.tensor.matmul(pt[:], ones1[:], t_row[:, ch * 512:(ch + 1) * 512])
                nc.any.tensor_copy(t_sb[:, ch * 512:(ch + 1) * 512], pt[:])

            # rowsum partials (128 x 8jc) per it
            rs_sb = cst.tile([128, NIT * NJC], F32, tag="rssb")

            # Z^T accumulators: 2 f-chunks x (128f x 1024i) fp32 psum
            zps = []
            for fc in range(2):
                zp = zpool.tile([128, RB], F32, tag=f"zps{fc}", name=f"zps{fc}")
                zps.append(zp)

            for jc in range(NJC):
                mf = []
                for it in range(NIT):
                    m_t = mfp.tile([128, JC], F32, tag="mf")
                    nc.sync.dma_start(
                        m_t[:],
                        m_d[it * 128:(it + 1) * 128, jc * JC:(jc + 1) * JC],
                    )
                    mf.append(m_t)
                p_tiles = []
                for it in range(NIT):
                    mb = chp.tile([128, JC], F16, tag="mb")
                    nc.gpsimd.tensor_copy(mb[:], mf[it][:])
                    e_t = chp.tile([128, JC], F16, tag="et")
                    nc.scalar.activation(
                        e_t[:], t_sb[:, jc * JC:(jc + 1) * JC], AF.Prelu,
                        bias=s_sb[:, it, :], scale=1.0, alpha=ALPHA,
                    )
                    v_t = chp.tile([128, JC], F16, tag="vt")
                    nc.vector.tensor_tensor(v_t[:], mb[:], e_t[:], ALU.mult)
                    z_t = chp.tile([128, JC], F16, tag="zt")
                    nc.gpsimd.tensor_scalar(
                        z_t[:], mb[:], 0.0, NEG, ALU.is_le, ALU.mult
                    )
                    w_t = chp.tile([128, JC], F16, tag="wt")
                    nc.vector.tensor_tensor(w_t[:], v_t[:], z_t[:], ALU.add)
                    p_t = ppool.tile([128, JC], F16, tag="pt")
                    nc.scalar.activation(
                        p_t[:], w_t[:], AF.Exp,
                        accum_out=rs_sb[:, it * NJC + jc: it * NJC + jc + 1],
                    )
                    p_tiles.append(p_t)

                for js in range(NJS):
                    ptr = trpool.tile([128, RB], F32, tag="pstr")
                    for it in range(NIT):
                        nc.tensor.matmul(
                            ptr[:, it * 128:(it + 1) * 128],
                            p_tiles[it][:, js * 128:(js + 1) * 128],
                            ident16[:],
                        )
                    pt_sb = misc.tile([128, RB], F16, tag="ptsb")
                    nc.any.tensor_copy(pt_sb[:], ptr[:])
                    j_abs = jc * NJS + js
                    for fc in range(2):
                        for ih in range(2):
                            nc.tensor.matmul(
                                zps[fc][:, ih * 512:(ih + 1) * 512],
                                h_sb[:, j_abs, fc * 128:(fc + 1) * 128],
                                pt_sb[:, ih * 512:(ih + 1) * 512],
                                start=(j_abs == 0),
                                stop=(j_abs == NJT - 1),
                            )

            # ---- finale
            zt_sb = []
            for fc in range(2):
                zt = cst.tile([128, RB], F32, tag=f"ztsb{fc}", name=f"ztsb{fc}")
                zt_sb.append(zt)
            for fc in range(2):
                nc.any.tensor_copy(zt_sb[fc][:], zps[fc][:])

            rec = cst.tile([128, NIT], F32, tag="rec")
            rtot = cst.tile([128, NIT], F32, tag="rtot")
            for it in range(NIT):
                nc.vector.tensor_reduce(
                    rtot[:, it:it + 1],
                    rs_sb[:, it * NJC:(it + 1) * NJC],
                    mybir.AxisListType.X, ALU.add,
                )
            nc.vector.reciprocal(rec[:], rtot[:])

            for it in range(NIT):
                pz = trpool.tile([128, FOUT], F32, tag="pstr")
                for fc in range(2):
                    nc.tensor.matmul(
                        pz[:, fc * 128:(fc + 1) * 128],
                        zt_sb[fc][:, it * 128:(it + 1) * 128],
                        ident32[:],
                    )
                z_out = misc.tile([128, FOUT], F32, tag="zout")
                nc.scalar.activation(
                    z_out[:], pz[:], AF.Sigmoid,
                    bias=0.0, scale=rec[:, it:it + 1],
                )
                nc.sync.dma_start(z_d[it * 128:(it + 1) * 128, :], z_out[:])
    nc.finalize()
    return nc


# ----------------------------------------------------------------- host glue

_CACHE = {}


def _get_kernels():
    if "h" not in _CACHE:
        _CACHE["h"] = build_h_kernel()
        _CACHE["main"] = build_main_kernel()
    return _CACHE["h"], _CACHE["main"]


def kernel(X, M, W_w, attn_w):
    X = np.ascontiguousarray(X, dtype=np.float32)
    M = np.ascontiguousarray(M, dtype=np.float32)
    W_w = np.ascontiguousarray(W_w, dtype=np.float32)
    attn_w = np.ascontiguousarray(attn_w, dtype=np.float32).reshape(1, 2 * FOUT)

    nc_h, nc_main = _get_kernels()
    cores = list(range(NCORES))

    in1 = [
        {"xc": X[c * RB:(c + 1) * RB], "w": W_w, "attn": attn_w}
        for c in cores
    ]
    r1 = run_bass_kernel_spmd(nc_h, in1, cores).results
    H = np.concatenate([r1[c]["hc"] for c in cores], axis=0)      # fp16
    t = np.concatenate([r1[c]["tc"] for c in cores], axis=0)      # (N,1) f32
    t_row = np.ascontiguousarray(t.reshape(1, N))

    in2 = [
        {
            "mc": M[c * RB:(c + 1) * RB],
            "hf": H,
            "tf": t_row,
            "sc": r1[c]["sc"],
        }
        for c in cores
    ]
    r2 = run_bass_kernel_spmd(nc_main, in2, cores).results
    Z = np.concatenate([r2[c]["zc"] for c in cores], axis=0)
    return Z.astype(np.float32)


# revision 23
# speedup vs baseline: 20462.8746x; 20462.8746x over previous
"""DenseStructuralGAT layer on 8 Trainium2 NeuronCores.

Row-parallel sharding: core c owns rows [c*1024, (c+1)*1024) of the
8192x8192 attention problem.

Launch 1 (tiny): each core computes its slice H_c = X_c @ W^T (fp16 out),
s_c = X_c @ (W^T a_l), t_c = X_c @ (W^T a_r)  (fp32).
Host concatenates H (fp16) and t (fp32) - pure data movement.

Launch 2 (main): per core, over its 1024x8192 block of M:
  e     = prelu(t_j + s_i, alpha=0.2)            [ACT, one pass, fp16]
  v     = fp16(M) * e                            [DVE tensor_tensor]
  z     = (fp16(M) <= 0) * -50                   [GPSIMD dual-op tensor_scalar]
  w     = v + z                                  [DVE tensor_tensor]
  p     = exp(w), rowsum += accum_out            [ACT, fp16 out]
  pT    = transpose(p) via PE identity matmuls   [PE -> PSUM -> SBUF fp16]
  Z^T  += H_tile^T @ pT                          [PE, fp32 PSUM accum]
  Z     = sigmoid(Z^T.T * (1/rowsum))            [PE transpose + ACT]

Masked entries (M==0) get logits=-50 -> p ~ 2e-22, i.e. softmax-identical
to the reference's -inf masking. Rows with no neighbours do not occur for
this input distribution (checked in test.py; p(no-neighbour) ~ 0.9^8192).
"""

import numpy as np
import ml_dtypes

import concourse.bacc as bacc
import concourse.mybir as mybir
import concourse.tile as tile
from concourse.bass_utils import run_bass_kernel_spmd
from concourse.masks import make_identity

N, FIN, FOUT = 8192, 512, 256
NCORES = 8
RB = N // NCORES          # 1024 rows per core
NIT = RB // 128           # 8 i-tiles per core
NJC = 8                   # j-chunks
JC = N // NJC             # 1024 columns per chunk
NJS = JC // 128           # 8 j-subtiles per chunk
NJT = N // 128            # 64 j-tiles total
NEG = -50.0
ALPHA = 0.2

F32 = mybir.dt.float32
F16 = mybir.dt.float16
AF = mybir.ActivationFunctionType
ALU = mybir.AluOpType


# ----------------------------------------------------------------- launch 1

def build_h_kernel():
    nc = bacc.Bacc()
    x_d = nc.dram_tensor("xc", [RB, FIN], F32, kind="ExternalInput")
    w_d = nc.dram_tensor("w", [FOUT, FIN], F32, kind="ExternalInput")
    a_d = nc.dram_tensor("attn", [1, 2 * FOUT], F32, kind="ExternalInput")
    h_d = nc.dram_tensor("hc", [RB, FOUT], F16, kind="ExternalOutput")
    s_d = nc.dram_tensor("sc", [RB, 1], F32, kind="ExternalOutput")
    t_d = nc.dram_tensor("tc", [RB, 1], F32, kind="ExternalOutput")

    with tile.TileContext(nc) as tc:
        with (
            tc.tile_pool(name="sb", bufs=1) as sb,
            tc.tile_pool(name="ps", bufs=2, space="PSUM") as ps,
            tc.tile_pool(name="psacc", bufs=2, space="PSUM") as psacc,
        ):
            ident = sb.tile([128, 128], F32, tag="ident")
            make_identity(nc, ident[:])
            one11 = sb.tile([1, 1], F32, tag="one")
            nc.vector.memset(one11[:], 1.0)

            x_sb = sb.tile([128, NIT, FIN], F32, tag="xsb")
            nc.sync.dma_start(
                x_sb[:], x_d[:].rearrange("(it p) k -> p it k", p=128)
            )
            w_sb = sb.tile([128, 2, FIN], F32, tag="wsb")
            nc.sync.dma_start(
                w_sb[:], w_d[:].rearrange("(ft p) k -> p ft k", p=128)
            )
            a_sb = sb.tile([1, 2 * FOUT], F32, tag="asb")
            nc.sync.dma_start(a_sb[:], a_d[:])

            # attention vector chunks as columns: 4 chunks of 128
            # [a_l0 a_l1 | a_r0 a_r1]
            a_cols = sb.tile([128, 4], F32, tag="acols")
            for h in range(4):
                pa = ps.tile([128, 1], F32, tag="pt")
                nc.tensor.matmul(pa[:], a_sb[0:1, 128 * h:128 * (h + 1)], one11[:])
                nc.any.tensor_copy(a_cols[:, h:h + 1], pa[:])

            # W^T tiles: WT[kt] = (128k x 256f)
            wt_sb = sb.tile([128, 4 * FOUT], F32, tag="wtsb")
            for kt in range(4):
                for ft in range(2):
                    pw = ps.tile([128, 128], F32, tag="pt")
                    nc.tensor.matmul(
                        pw[:],
                        w_sb[:, ft, kt * 128:(kt + 1) * 128],
                        ident[:],
                    )
                    nc.any.tensor_copy(
                        wt_sb[:, kt * FOUT + ft * 128: kt * FOUT + (ft + 1) * 128],
                        pw[:],
                    )

            # w_s / w_t columns per k-tile: wst[kc] = (128k x 2)
            wst = sb.tile([128, 8], F32, tag="wst")
            for kc in range(4):
                pst = psacc.tile([128, 2], F32, tag="pstc")
                for ft in range(2):
                    rhs = sb.tile([128, 2], F32, tag="arhs")
                    nc.vector.tensor_copy(rhs[:, 0:1], a_cols[:, ft:ft + 1])
                    nc.vector.tensor_copy(rhs[:, 1:2], a_cols[:, 2 + ft:3 + ft])
                    nc.tensor.matmul(
                        pst[:],
                        w_sb[:, ft, kc * 128:(kc + 1) * 128],
                        rhs[:],
                        start=(ft == 0),
                        stop=(ft == 1),
                    )
                nc.any.tensor_copy(wst[:, 2 * kc: 2 * kc + 2], pst[:])

            # X^T tiles: XT[kt] = (128k x 1024i)
            xt_sb = sb.tile([128, 4 * RB], F32, tag="xtsb")
            for it in range(NIT):
                for kt in range(4):
                    px = ps.tile([128, 128], F32, tag="pt")
                    nc.tensor.matmul(
                        px[:],
                        x_sb[:, it, kt * 128:(kt + 1) * 128],
                        ident[:],
                    )
                    nc.any.tensor_copy(
                        xt_sb[:, kt * RB + it * 128: kt * RB + (it + 1) * 128],
                        px[:],
                    )

            # H_c, s_c, t_c
            for it in range(NIT):
                ph = psacc.tile([128, FOUT], F32, tag="ph")
                pstc = psacc.tile([128, 2], F32, tag="pstc")
                for kt in range(4):
                    lhs = xt_sb[:, kt * RB + it * 128: kt * RB + (it + 1) * 128]
                    nc.tensor.matmul(
                        ph[:], lhs, wt_sb[:, kt * FOUT:(kt + 1) * FOUT],
                        start=(kt == 0), stop=(kt == 3),
                    )
                    nc.tensor.matmul(
                        pstc[:], lhs, wst[:, 2 * kt: 2 * kt + 2],
                        start=(kt == 0), stop=(kt == 3),
                    )
                hb = sb.tile([128, FOUT], F16, tag="hb")
                stb = sb.tile([128, 2], F32, tag="stb")
                nc.any.tensor_copy(hb[:], ph[:])
                nc.any.tensor_copy(stb[:], pstc[:])
                nc.sync.dma_start(h_d[it * 128:(it + 1) * 128, :], hb[:])
                nc.sync.dma_start(s_d[it * 128:(it + 1) * 128, :], stb[:, 0:1])
                nc.sync.dma_start(t_d[it * 128:(it + 1) * 128, :], stb[:, 1:2])
    nc.finalize()
    return nc


# ----------------------------------------------------------------- launch 2

def build_main_kernel(reps=1, skip_pe=False, skip_chain=False, pool_engine='dve', dynamic=False):
    nc = bacc.Bacc()
    m_d = nc.dram_tensor("mc", [RB, N], F32, kind="ExternalInput")
    h_d = nc.dram_tensor("hf", [N, FOUT], F16, kind="ExternalInput")
    t_d = nc.dram_tensor("tf", [1, N], F32, kind="ExternalInput")
    s_d = nc.dram_tensor("sc", [RB, 1], F32, kind="ExternalInput")
    z_d = nc.dram_tensor("zc", [RB, FOUT], F32, kind="ExternalOutput")

    with tile.TileContext(nc) as tc:
        with (
            tc.tile_pool(name="const", bufs=1) as cst,
            tc.tile_pool(name="zps", bufs=1, space="PSUM") as zpool,
            tc.tile_pool(name="trps", bufs=2, space="PSUM") as trpool,
            tc.tile_pool(name="mf", bufs=10) as mfp,
            tc.tile_pool(name="chain", bufs=6) as chp,
            tc.tile_pool(name="pchain", bufs=16) as ppool,
            tc.tile_pool(name="misc", bufs=4) as misc,
        ):
            ident16 = cst.tile([128, 128], F16, tag="id16")
            make_identity(nc, ident16[:])
            ident32 = cst.tile([128, 128], F32, tag="id32")
            make_identity(nc, ident32[:])
            ones1 = cst.tile([1, 128], F32, tag="ones1")
            nc.vector.memset(ones1[:], 1.0)
            onescol = cst.tile([128, 1], F16, tag="onescol")
            nc.vector.memset(onescol[:], 1.0)

            # full H as lhsT tiles: (128j x 64jt*256f) fp16
            h_sb = cst.tile([128, NJT, FOUT], F16, tag="hsb")
            nc.sync.dma_start(
                h_sb[:], h_d[:].rearrange("(jt p) f -> p jt f", p=128)
            )
            # s columns (128 x 8it)
            s_sb = cst.tile([128, NIT, 1], F32, tag="ssb")
            nc.sync.dma_start(
                s_sb[:], s_d[:].rearrange("(it p) o -> p it o", p=128)
            )
            # T_bcast (128 x 8192) fp16 via K=1 ones matmul; t loaded in chunks
            t_sb = cst.tile([128, N], F16, tag="tsb")
            for ch in range(N // 512):
                t_chunk = misc.tile([1, 512], F32, tag="tchunk")
                nc.sync.dma_start(t_chunk[:], t_d[:, ch * 512:(ch + 1) * 512])
                pt = trpool.tile([128, 512], F32, tag="pstr")
                nc.tensor.matmul(pt[:], ones1[:], t_chunk[:])
                nc.vector.tensor_copy(t_sb[:, ch * 512:(ch + 1) * 512], pt[:])



            # Z^T accumulators: 2 f-chunks x (128f x 1024i) fp32 psum
            zps = []
            rsT = None
            if not skip_pe:
                for fc in range(2):
                    zp = zpool.tile([128, RB], F32, tag=f"zps{fc}", name=f"zps{fc}")
                    zps.append(zp)
                rsT = zpool.tile([1, RB], F32, tag="rsT", name="rsT")

            rep_ctx = tc.For_i(0, reps, 1) if dynamic else None
            if rep_ctx is not None:
                rep_ctx.__enter__()
            for rep in range(1 if dynamic else reps):
              for jc in range(NJC):
                mf = []
                for it in range(NIT):
                    m_t = mfp.tile([128, JC], F32, tag="mf")
                    dma_eng = nc.sync if it % 2 == 0 else nc.scalar
                    dma_eng.dma_start(
                        m_t[:],
                        m_d[it * 128:(it + 1) * 128, jc * JC:(jc + 1) * JC],
                    )
                    mf.append(m_t)
                p_tiles = []
                if skip_chain:
                    for it in range(NIT):
                        p_t = ppool.tile([128, JC], F16, tag="pt")
                        if jc == 0 and rep == 0:
                            nc.vector.memset(p_t[:], 0.01)
                        p_tiles.append(p_t)
                else:
                    for it in range(NIT):
                        e_t = chp.tile([128, JC], F16, tag="et")
                        nc.scalar.activation(
                            e_t[:], t_sb[:, jc * JC:(jc + 1) * JC], AF.Prelu,
                            bias=s_sb[:, it, :], scale=1.0, alpha=ALPHA,
                        )
                        v_t = chp.tile([128, JC], F16, tag="vt")
                        nc.vector.tensor_tensor(v_t[:], mf[it][:], e_t[:], ALU.mult)
                        z_t = chp.tile([128, JC], F16, tag="zt")
                        nc.vector.tensor_scalar(
                            z_t[:], v_t[:], 0.0, NEG, ALU.is_equal, ALU.mult
                        )
                        p_t = ppool.tile([128, JC], F16, tag="pt")
                        nc.vector.tensor_tensor(p_t[:], v_t[:], z_t[:], ALU.add)
                        p_tiles.append(p_t)

                for js in range(NJS if not skip_pe else 0):
                    j_abs = jc * NJS + js
                    pt_sb = misc.tile([128, RB], F16, tag="ptsb")
                    for ih in range(2):
                        ptr = trpool.tile([128, 512], F32, tag="pstr")
                        for it4 in range(4):
                            it = ih * 4 + it4
                            nc.tensor.matmul(
                                ptr[:, it4 * 128:(it4 + 1) * 128],
                                p_tiles[it][:, js * 128:(js + 1) * 128],
                                ident16[:],
                            )
                        nc.scalar.activation(
                            pt_sb[:, ih * 512:(ih + 1) * 512], ptr[:], AF.Exp,
                        )
                        nc.tensor.matmul(
                            rsT[:, ih * 512:(ih + 1) * 512],
                            onescol[:],
                            pt_sb[:, ih * 512:(ih + 1) * 512],
                            start=(j_abs == 0),
                            stop=(j_abs == NJT - 1),
                        )
                    for fc in range(2):
                        for ih in range(2):
                            nc.tensor.matmul(
                                zps[fc][:, ih * 512:(ih + 1) * 512],
                                h_sb[:, j_abs, fc * 128:(fc + 1) * 128],
                                pt_sb[:, ih * 512:(ih + 1) * 512],
                                start=(j_abs == 0),
                                stop=(j_abs == NJT - 1),
                            )

            if rep_ctx is not None:
                rep_ctx.__exit__(None, None, None)
            # ---- finale
            zt_sb = []
            for fc in range(2):
                zt = cst.tile([128, RB], F32, tag=f"ztsb{fc}", name=f"ztsb{fc}")
                zt_sb.append(zt)
            for fc in range(2):
                if skip_pe:
                    nc.vector.memset(zt_sb[fc][:], 0.5)
                else:
                    nc.any.tensor_copy(zt_sb[fc][:], zps[fc][:])

            rec = cst.tile([128, NIT], F32, tag="rec")
            rtot = cst.tile([128, NIT], F32, tag="rtot")
            if skip_pe or skip_chain:
                nc.vector.memset(rtot[:], 1.0)
            else:
                rs_row = cst.tile([1, RB], F32, tag="rsrow")
                nc.vector.tensor_copy(rs_row[:], rsT[:])
                one11b = cst.tile([1, 1], F32, tag="one11b")
                nc.vector.memset(one11b[:], 1.0)
                for it in range(NIT):
                    prs = trpool.tile([128, 1], F32, tag="pstr")
                    nc.tensor.matmul(
                        prs[:], rs_row[0:1, it * 128:(it + 1) * 128], one11b[:])
                    nc.vector.tensor_copy(rtot[:, it:it + 1], prs[:])
            nc.vector.reciprocal(rec[:], rtot[:])

            for it in range(NIT):
                pz = trpool.tile([128, FOUT], F32, tag="pstr")
                for fc in range(2):
                    nc.tensor.matmul(
                        pz[:, fc * 128:(fc + 1) * 128],
                        zt_sb[fc][:, it * 128:(it + 1) * 128],
                        ident32[:],
                    )
                z_out = misc.tile([128, FOUT], F32, tag="zout")
                nc.scalar.activation(
                    z_out[:], pz[:], AF.Sigmoid,
                    bias=0.0, scale=rec[:, it:it + 1],
                )
                nc.sync.dma_start(z_d[it * 128:(it + 1) * 128, :], z_out[:])
    nc.finalize()
    return nc


# ----------------------------------------------------------------- host glue

_CACHE = {}


def _get_kernels():
    if "h" not in _CACHE:
        _CACHE["h"] = build_h_kernel()
        _CACHE["main"] = build_main_kernel()
    return _CACHE["h"], _CACHE["main"]


def kernel(X, M, W_w, attn_w):
    X = np.ascontiguousarray(X, dtype=np.float32)
    M = np.ascontiguousarray(M, dtype=np.float32)
    W_w = np.ascontiguousarray(W_w, dtype=np.float32)
    attn_w = np.ascontiguousarray(attn_w, dtype=np.float32).reshape(1, 2 * FOUT)

    nc_h, nc_main = _get_kernels()
    cores = list(range(NCORES))

    in1 = [
        {"xc": X[c * RB:(c + 1) * RB], "w": W_w, "attn": attn_w}
        for c in cores
    ]
    r1 = run_bass_kernel_spmd(nc_h, in1, cores).results
    H = np.concatenate([r1[c]["hc"] for c in cores], axis=0)      # fp16
    t = np.concatenate([r1[c]["tc"] for c in cores], axis=0)      # (N,1) f32
    t_row = np.ascontiguousarray(t.reshape(1, N))

    in2 = [
        {
            "mc": M[c * RB:(c + 1) * RB],
            "hf": H,
            "tf": t_row,
            "sc": r1[c]["sc"],
        }
        for c in cores
    ]
    r2 = run_bass_kernel_spmd(nc_main, in2, cores).results
    Z = np.concatenate([r2[c]["zc"] for c in cores], axis=0)
    return Z.astype(np.float32)
